# revision 15
# baseline (speedup 1.0000x reference)
"""Gumbel top-k (sequential masking) Trainium2 kernel, v4.

B=64 rows, N=16384, K=16 sequential top-1+mask steps; outputs st
(one-hot) and softs, each [K, B, N] f32 (softs emitted bf16, st u8).
Data-parallel: 8 rows/core x 8 cores; row = 16 partitions x 1024.
DRAM outputs partition-major [P, K*1024]; host transposes back.

Structure:
  - st = zero-fill (one 2MB DMA from a zeroed SBUF block) + 128
    scattered one-bytes ([P,1] indirect DMA).
  - softs plane j = e_src * (1/S_j), bf16 out, split ACT/DVE.
    Tree: e0 -> e4 -> e8 -> e12 -> e13 -> e14 -> e15 (6 match_replace).
    Planes j in {1,2,3, 5,6,7, 9,10,11} are group-masked; their stale
    positions are zeroed in DRAM by two [P,1] indirect scatters
    (q-packed items; pad slots write an idempotent 0 to plane15's
    rank-14 cell). Group 3 planes are exact - no fix-up after the
    last group DMA.
  - Offsets: each partition stages premultiplied candidate offsets
    (p*16384 + half*512 + max_index) and rank-ordered slot pointers
    into one DRAM table; PLAIN strided-AP readback DMAs (a 2-level
    [1024r + 65q + base] access pattern) extract per-partition
    diagonals (host constant maps baked into column layout), then one
    [P,1] indirect gather per consumer resolves the final offsets.
  - DRAM W-A-W/R-A-W ordering via Tile-visible tokens: a memset into
    a DMA's read region (waits completion), value-preserving rewrite
    into the consumer's tile.
  - 1/S_j split in three (S0 / ranks 1-7 / ranks 8-15) so early
    planes start as soon as possible.
"""

import numpy as np
from contextlib import ExitStack

import concourse.bacc as bacc
import concourse.bass as bass
import concourse.mybir as mybir
import concourse.tile as tile
from concourse.bass import AP
from concourse.bass_utils import run_bass_kernel_spmd

F32 = mybir.dt.float32
BF16 = mybir.dt.bfloat16
U8 = mybir.dt.uint8
U32 = mybir.dt.uint32
AF = mybir.ActivationFunctionType
OP = mybir.AluOpType

B, N, NCORES = 64, 16384, 8
R = B // NCORES
QP = 16
FREE = N // QP           # 1024
P = 128
H = FREE // 2
INV_TAU = 1.5
K16 = 16
KF = K16 * FREE
SW = 64                  # staging row width (u32 elements per partition)

# q-packed scatter items: (plane, rank) per q slot
ITEMS_A = [(1, 0), (2, 0), (2, 1), (3, 0), (3, 1), (3, 2),
           (5, 4), (6, 4), (6, 5), (7, 4), (7, 5), (7, 6)]
ITEMS_B = [(9, 8), (10, 8), (10, 9), (11, 8), (11, 9), (11, 10)]

ACT_PLANES = {0, 1, 3, 5, 6, 7, 9, 10, 11, 13, 15}

_module_cache = {}


def _host_consts():
    p = np.arange(P)
    q = p % 16
    cc = np.zeros((P, 8), np.uint32)
    cc[:, 0] = p * 16384
    cc[:, 1] = p * 16384 + 512
    cc[:, 2] = (p // 16) * 1024          # slotflat row base
    cc[:, 3] = q * 1024                  # st plane offset (j = q)
    ja = np.array([j for j, _ in ITEMS_A] + [15] * 4, np.uint32)
    jb = np.array([j for j, _ in ITEMS_B] + [15] * 10, np.uint32)
    cc[:, 4] = ja[q] * 1024
    cc[:, 5] = jb[q] * 1024
    return cc


def _build16():
    nc = bacc.Bacc("TRN2", target_bir_lowering=False, debug=False,
                   num_devices=NCORES)
    zc_d = nc.dram_tensor("zc", [P, FREE + 8], F32, kind="ExternalInput")
    softs_d = nc.dram_tensor("softs", [P * KF, 1], BF16, kind="ExternalOutput")
    st_d = nc.dram_tensor("st", [P * KF, 1], U8, kind="ExternalOutput")
    stg_d = nc.dram_tensor("stg", [P * SW, 1], U32, kind="Internal")

    softs_2d = softs_d.ap().rearrange("(p f) o -> p (f o)", p=P)
    st_2d = st_d.ap().rearrange("(p f) o -> p (f o)", p=P)
    stg_2d = stg_d.ap().rearrange("(p c) o -> p (c o)", p=P)

    # strided diagonal readbacks: partition p=(16r+q) reads element
    # base + 1024r + 65q  (= p*64 + base + q)
    def diag_ap(base):
        return AP(stg_d.ap().tensor, base, [[1024, 8], [65, 16]])

    with tile.TileContext(nc) as tc, ExitStack() as ctx:
        sp = ctx.enter_context(tc.tile_pool(name="sp", bufs=1))

        zext = sp.tile([P, FREE + 8], F32, tag="zext")
        z = zext[:, 0:FREE]
        cc = zext[:, FREE:FREE + 8].bitcast(U32)
        e0 = sp.tile([P, FREE], F32, tag="e0")
        etiles = {0: e0}
        for t in (4, 8, 12, 13, 14, 15):
            etiles[t] = sp.tile([P, FREE], F32, tag=f"e{t}", name=f"e{t}")
        softs_sb = sp.tile([P, KF], BF16, tag="softs_sb")
        stz = sp.tile([P, KF // 4], F32, tag="stz")
        sel = sp.tile([P, 18], F32, tag="sel")
        miu = sp.tile([P, 16], U32, tag="miu")
        scomb = sp.tile([P, SW], U32, tag="scomb")
        cand = sp.tile([P, 16 * 18], F32, tag="cand")
        S0 = sp.tile([P, 1], F32, tag="S0")
        vbr = sp.tile([P, 32], F32, tag="vbr")
        ec = sp.tile([P, 256], F32, tag="ec")
        c2 = sp.tile([P, 256], F32, tag="c2")
        slots = sp.tile([P, 16], U32, tag="slots")
        sadj = sp.tile([P, 64], U32, tag="sadj")
        rb = sp.tile([P, 3], U32, tag="rb")       # readback diag results
        ob = sp.tile([P, 3], U32, tag="ob")       # gathered premult values
        otmp = sp.tile([P, 3], U32, tag="otmp")
        offd = sp.tile([P, 3], U32, tag="offd")
        ones = sp.tile([P, 1], U8, tag="ones")
        zbf = sp.tile([P, 1], BF16, tag="zbf")
        padk = sp.tile([P, 48], F32, tag="padk")
        pfa0 = sp.tile([P, 8], F32, tag="pfa0")
        pfa1 = sp.tile([P, 8], F32, tag="pfa1")
        pfb0 = sp.tile([P, 8], F32, tag="pfb0")
        pfb1 = sp.tile([P, 8], F32, tag="pfb1")
        SSp = sp.tile([P, 16], F32, tag="SSp")
        SA = sp.tile([P, 1], F32, tag="SA")

        # ---- inputs (z halves; consts ride with half B) + st zero-fill
        nc.sync.dma_start(out=zext[:, 0:H], in_=zc_d.ap()[:, 0:H])
        nc.sync.dma_start(out=zext[:, H:FREE + 8],
                          in_=zc_d.ap()[:, H:FREE + 8])
        nc.gpsimd.memset(stz[:], 0.0)
        nc.sync.dma_start(out=st_2d, in_=stz[:].bitcast(U8))
        nc.gpsimd.memset(stz[:, 0:1], 0.0)       # token: zero-fill done

        # ---- exp + per-partition selection
        nc.scalar.activation(e0[:, 0:H], z[:, 0:H], AF.Exp, scale=INV_TAU,
                             accum_out=sel[:, 16:17])
        nc.scalar.activation(e0[:, H:FREE], z[:, H:FREE], AF.Exp,
                             scale=INV_TAU, accum_out=sel[:, 17:18])
        nc.vector.max(sel[:, 0:8], e0[:, 0:H])
        nc.vector.max_index(miu[:, 0:8], sel[:, 0:8], e0[:, 0:H])
        nc.vector.max(sel[:, 8:16], e0[:, H:FREE])
        nc.vector.max_index(miu[:, 8:16], sel[:, 8:16], e0[:, H:FREE])
        nc.gpsimd.tensor_tensor(scomb[:, 0:8], miu[:, 0:8],
                                cc[:, 0:1].to_broadcast([P, 8]), OP.add)
        nc.gpsimd.tensor_tensor(scomb[:, 8:16], miu[:, 8:16],
                                cc[:, 1:2].to_broadcast([P, 8]), OP.add)

        # ---- row-level selection
        for q in range(QP):
            nc.vector.stream_shuffle(cand[:, 18 * q:18 * q + 18], sel[:],
                                     [q] * 16 + [16 + q] * 16)
        gv = cand[:].rearrange("p (q c) -> p q c", c=18)
        nc.vector.tensor_reduce(S0[:], gv[:, :, 16:18],
                                axis=mybir.AxisListType.XY, op=OP.add)
        nc.vector.reciprocal(vbr[:, 16:17], S0[:])
        nc.vector.max(vbr[:, 0:8], gv[:, :, 0:16])
        nc.vector.tensor_copy(ec[:].rearrange("p (q j) -> p q j", j=16),
                              gv[:, :, 0:16])
        nc.vector.max_index(slots[:, 0:8], vbr[:, 0:8], ec[:])
        nc.vector.match_replace(c2[:], vbr[:, 0:8], ec[:], 0.0)
        nc.vector.max(vbr[:, 8:16], c2[:])
        nc.vector.max_index(slots[:, 8:16], vbr[:, 8:16], c2[:])
        # slot s -> flat pointer p_src*64 + c = 1024r + 4s - 3*(s&15)
        hp = tc.high_priority()
        hp.__enter__()
        nc.vector.tensor_scalar(sadj[:, 0:16], slots[:], 2, None,
                                OP.logical_shift_left)
        nc.vector.tensor_scalar(sadj[:, 16:32], slots[:], 15, None,
                                OP.bitwise_and)
        nc.vector.tensor_scalar(sadj[:, 32:48], sadj[:, 16:32], 3, None,
                                OP.mult)
        nc.vector.tensor_tensor(sadj[:, 48:64], sadj[:, 0:16],
                                sadj[:, 32:48], OP.subtract)
        nc.vector.tensor_tensor(scomb[:, 16:32], sadj[:, 48:64],
                                cc[:, 2:3].to_broadcast([P, 16]), OP.add)
        # item columns (constant rank maps) + idempotent pads (rank 14)
        nc.vector.tensor_copy(scomb[:, 32:33], scomb[:, 16:17])
        nc.vector.tensor_copy(scomb[:, 33:35], scomb[:, 16:18])
        nc.vector.tensor_copy(scomb[:, 35:38], scomb[:, 16:19])
        nc.vector.tensor_copy(scomb[:, 38:39], scomb[:, 20:21])
        nc.vector.tensor_copy(scomb[:, 39:41], scomb[:, 20:22])
        nc.vector.tensor_copy(scomb[:, 41:44], scomb[:, 20:23])
        nc.vector.tensor_copy(scomb[:, 44:48],
                              scomb[:, 30:31].to_broadcast([P, 4]))
        nc.vector.tensor_copy(scomb[:, 48:49], scomb[:, 24:25])
        nc.vector.tensor_copy(scomb[:, 49:51], scomb[:, 24:26])
        nc.vector.tensor_copy(scomb[:, 51:54], scomb[:, 24:27])
        nc.vector.tensor_copy(scomb[:, 54:64],
                              scomb[:, 30:31].to_broadcast([P, 10]))
        hp.__exit__(None, None, None)

        # ---- staging + ordered readbacks
        nc.sync.dma_start(out=stg_2d, in_=scomb[:])
        nc.gpsimd.memset(scomb[:, 63:64], 0)      # token: staging done
        for x in range(3):
            nc.gpsimd.tensor_copy(rb[:, x:x + 1], scomb[:, 63:64])
        nc.sync.dma_start(out=rb[:, 0:1], in_=diag_ap(16))
        nc.sync.dma_start(out=rb[:, 1:2], in_=diag_ap(32))
        nc.sync.dma_start(out=rb[:, 2:3], in_=diag_ap(48))

        # ---- 1/S_j (split: ranks 1-7, then 8-15)
        pfa = [pfa0, pfa1]
        nc.gpsimd.tensor_copy(pfa[0][:], vbr[:, 0:8])
        cur = 0
        for sh in (1, 2, 4):
            nxt = 1 - cur
            nc.gpsimd.tensor_copy(pfa[nxt][:, 0:sh], pfa[cur][:, 0:sh])
            nc.gpsimd.tensor_tensor(pfa[nxt][:, sh:8], pfa[cur][:, sh:8],
                                    pfa[cur][:, 0:8 - sh], OP.add)
            cur = nxt
        nc.gpsimd.tensor_scalar(SSp[:, 1:8], pfa[cur][:, 0:7], -1.0, S0[:],
                                OP.mult, OP.add)
        nc.vector.reciprocal(vbr[:, 17:24], SSp[:, 1:8])
        nc.gpsimd.tensor_scalar(SA[:], pfa[cur][:, 7:8], -1.0, S0[:],
                                OP.mult, OP.add)
        pfb = [pfb0, pfb1]
        nc.gpsimd.tensor_copy(pfb[0][:], vbr[:, 8:16])
        cur = 0
        for sh in (1, 2, 4):
            nxt = 1 - cur
            nc.gpsimd.tensor_copy(pfb[nxt][:, 0:sh], pfb[cur][:, 0:sh])
            nc.gpsimd.tensor_tensor(pfb[nxt][:, sh:8], pfb[cur][:, sh:8],
                                    pfb[cur][:, 0:8 - sh], OP.add)
            cur = nxt
        nc.gpsimd.tensor_copy(SSp[:, 8:9], SA[:])
        nc.gpsimd.tensor_scalar(SSp[:, 9:16], pfb[cur][:, 0:7], -1.0, SA[:],
                                OP.mult, OP.add)
        nc.vector.reciprocal(vbr[:, 24:32], SSp[:, 8:16])

        # ---- mr tree keys
        nc.vector.memset(padk[:], -1.0)
        nc.vector.tensor_copy(padk[:, 0:4], vbr[:, 0:4])
        nc.vector.tensor_copy(padk[:, 8:12], vbr[:, 4:8])
        nc.vector.tensor_copy(padk[:, 16:20], vbr[:, 8:12])
        nc.vector.tensor_copy(padk[:, 24:25], vbr[:, 12:13])
        nc.vector.tensor_copy(padk[:, 32:34], vbr[:, 12:14])
        nc.vector.tensor_copy(padk[:, 40:43], vbr[:, 12:15])

        # ---- planes + tree + group DMAs, interleaved by dependency
        def plane(j, src):
            dst = softs_sb[:, j * FREE:(j + 1) * FREE]
            scl = vbr[:, 16 + j:17 + j]
            if j in ACT_PLANES:
                nc.scalar.activation(dst, src[:], AF.Copy, scale=scl)
            else:
                nc.vector.tensor_scalar(dst, src[:], scl, None, OP.mult)

        mr_edges = {4: (0, 0), 8: (4, 8), 12: (8, 16), 13: (12, 24),
                    14: (12, 32), 15: (12, 40)}

        for g in range(4):
            base = 4 * g
            if base in mr_edges:
                src_t, koff = mr_edges[base]
                nc.vector.match_replace(etiles[base][:],
                                        padk[:, koff:koff + 8],
                                        etiles[src_t][:], 0.0)
            for j in range(base, base + 4):
                if g < 3:
                    plane(j, etiles[base])
                else:
                    if j > 12 and j in mr_edges:
                        src_t, koff = mr_edges[j]
                        nc.vector.match_replace(etiles[j][:],
                                                padk[:, koff:koff + 8],
                                                etiles[src_t][:], 0.0)
                    plane(j, etiles[j])
            if g < 3:
                nc.sync.dma_start(
                    out=softs_2d[:, base * FREE:(base + 4) * FREE],
                    in_=softs_sb[:, base * FREE:(base + 4) * FREE])

        # ---- indirect block (st chain first; group tokens placed just
        # before their consumer so nothing queues behind a later group)
        nc.vector.memset(ones[:], 1)
        nc.vector.memset(zbf[:], 0.0)
        tok = [stz[:, 0:1].bitcast(U32),
               softs_sb[:, 0:2].bitcast(U32),
               softs_sb[:, 4 * FREE:4 * FREE + 2].bitcast(U32),
               softs_sb[:, 8 * FREE:8 * FREE + 2].bitcast(U32)]
        for x in range(3):
            nc.gpsimd.indirect_dma_start(
                out=ob[:, x:x + 1], out_offset=None, in_=stg_d.ap(),
                in_offset=bass.IndirectOffsetOnAxis(ap=rb[:, x:x + 1],
                                                    axis=0))
        # st: ob0 + q*1024 + 0*tok_z
        nc.gpsimd.tensor_tensor(otmp[:, 0:1], ob[:, 0:1], cc[:, 3:4], OP.add)
        nc.gpsimd.tensor_tensor(offd[:, 0:1], otmp[:, 0:1], tok[0], OP.add)
        nc.gpsimd.indirect_dma_start(
            out=st_d.ap(),
            out_offset=bass.IndirectOffsetOnAxis(ap=offd[:, 0:1], axis=0),
            in_=ones[:], in_offset=None)
        # soft A (groups 0,1): ob1 + joffA + 0*tok_g0 + 0*tok_g1
        nc.gpsimd.memset(softs_sb[:, 0:2], 0.0)
        nc.gpsimd.memset(softs_sb[:, 4 * FREE:4 * FREE + 2], 0.0)
        nc.gpsimd.tensor_tensor(otmp[:, 1:2], ob[:, 1:2], cc[:, 4:5], OP.add)
        nc.gpsimd.tensor_tensor(otmp[:, 1:2], otmp[:, 1:2], tok[1], OP.add)
        nc.gpsimd.tensor_tensor(offd[:, 1:2], otmp[:, 1:2], tok[2], OP.add)
        nc.gpsimd.indirect_dma_start(
            out=softs_d.ap(),
            out_offset=bass.IndirectOffsetOnAxis(ap=offd[:, 1:2], axis=0),
            in_=zbf[:], in_offset=None)
        # soft B (group 2): ob2 + joffB + 0*tok_g2
        nc.gpsimd.memset(softs_sb[:, 8 * FREE:8 * FREE + 2], 0.0)
        nc.gpsimd.tensor_tensor(otmp[:, 2:3], ob[:, 2:3], cc[:, 5:6], OP.add)
        nc.gpsimd.tensor_tensor(offd[:, 2:3], otmp[:, 2:3], tok[3], OP.add)
        nc.gpsimd.indirect_dma_start(
            out=softs_d.ap(),
            out_offset=bass.IndirectOffsetOnAxis(ap=offd[:, 2:3], axis=0),
            in_=zbf[:], in_offset=None)
        nc.sync.dma_start(out=softs_2d[:, 12 * FREE:16 * FREE],
                          in_=softs_sb[:, 12 * FREE:16 * FREE])
    nc.compile()
    return nc


def kernel(logits, gumbel, k, trace=False):
    K = int(k)
    logits = np.ascontiguousarray(logits, dtype=np.float32)
    gumbel = np.ascontiguousarray(gumbel, dtype=np.float32)
    if K == 0:
        empty = np.zeros((0, B, N), dtype=np.float32)
        return empty, empty.copy()
    assert K == 16, f"kernel supports k=16 only, got {K}"
    assert logits.shape == (B, N) and gumbel.shape == (B, N)

    if K not in _module_cache:
        _module_cache[K] = _build16()
    nc = _module_cache[K]

    cc = _host_consts().view(np.float32)
    z_full = logits + gumbel
    in_maps = []
    for c in range(NCORES):
        sl = slice(c * R, (c + 1) * R)
        zc = np.concatenate([z_full[sl].reshape(P, FREE), cc], axis=1)
        in_maps.append({"zc": np.ascontiguousarray(zc)})

    res = run_bass_kernel_spmd(nc, in_maps, core_ids=list(range(NCORES)),
                               trace=trace)

    st = np.empty((K, B, N), dtype=np.float32)
    softs = np.empty((K, B, N), dtype=np.float32)
    for c in range(NCORES):
        sl = slice(c * R, (c + 1) * R)
        s = res.results[c]["softs"].reshape(R, QP, K, FREE)
        softs[:, sl, :] = np.transpose(s.astype(np.float32), (2, 0, 1, 3)) \
            .reshape(K, R, N)
        t = res.results[c]["st"].reshape(R, QP, K, FREE)
        st[:, sl, :] = np.transpose(t, (2, 0, 1, 3)).reshape(K, R, N) \
            .astype(np.float32)

    if trace:
        kernel.last_exec_time_ns = res.exec_time_ns
        kernel.last_results = res
    return st, softs
